# revision 34
# baseline (speedup 1.0000x reference)
# Causal self-attention (B=8, T=1024, C=1024, H=16, D=64) on 8 trn2 NeuronCores.
# Sharding: data-parallel over batch — core i computes batch element i entirely
# (weights replicated, no collectives).
#
# v3 schedule notes: the PE HAM clock gate re-throttles to 1.2 GHz after ~3.4us
# of contiguous idle, and transpose-mode matmuls don't count as PE activity.
#   - constants (identity/masks) are built on GpSimd BEFORE the big DMA issue
#     train (GpSimd both issues DMAs and builds constants, in order),
#   - dummy identity matmuls warm the PE during the initial x DMA,
#   - x transposes are regular matmuls (lhsT=x_block, rhs=identity) with
#     4-wide batched PSUM->SBUF casts,
#   - qk-projection groups, second-half v-projection units and partial output
#     projection (head chunks 0-5) are distributed through the attention phase
#     from a deadline-ordered filler queue so the PE never idles long enough
#     to re-throttle,
#   - softmax denominator: reciprocal straight off PSUM row 64 (no l copy),
#     K=1 matmul broadcast, DVE multiply.
#
# Per-core pipeline (all matmuls bf16 inputs, fp32 PSUM accumulation):
#   0. x [T,C] --cast-dma--> bf16, transpose via PE -> xT [C,T]
#   1. v[t,j] per half jvt (heads 8*jvt..) with a ones column (width 65) so
#      attn@v also yields the softmax denominator in row 64.
#   2. per head-pair hp: qkT[j,t] (lhsT=W column slice, rhs=xT), then per
#      (head, 512-query tile): scores sT[j,i] (K=64, two heads in row groups
#      0/64 run concurrently), merged diag-block mask add (DVE), exp on ACT
#      (scale=1/8) -> pT bf16, attn@v (M=65), reciprocal+bcast+mult -> oT.
#   3. out[t,c]: lhsT=oT chunk, rhs=W_out; head chunks 0-5 pre-accumulated
#      into bf16 partials during hp6/7, chunks 6-7 + partial + bias at the end.

import numpy as np
from contextlib import ExitStack

import concourse.bass as bass
import concourse.bacc as bacc
import concourse.mybir as mybir
import concourse.tile as tile
from concourse import bass_utils
from concourse.masks import make_identity

FP32 = mybir.dt.float32
BF16 = mybir.dt.bfloat16
FP8 = mybir.dt.float8e4
# wqk8 jt consumption order (head-pair major)
JT_ORDER = [0, 8, 1, 9, 2, 10, 3, 11, 4, 12, 5, 13, 6, 14, 7, 15]

B, T, C = 8, 1024, 1024
H, D = 16, 64
N_CORES = 8
MASK_VAL = -1e4  # pre-scale additive mask; exp(0.125 * (s + MASK_VAL)) == 0.0
CCH = C // 128   # 8 contraction chunks of 128
TCH = T // 128   # 8 token chunks of 128
N_WARMUP = 40    # dummy matmuls to warm the PE clock gate during x DMA


def build_nc():
    nc = bacc.Bacc("TRN2", debug=False, num_devices=N_CORES)

    x_d = nc.dram_tensor("x_b", [T, C], BF16, kind="ExternalInput").ap()
    wq8_d = nc.dram_tensor("w_qk8", [128, 16 * 1024], FP8, kind="ExternalInput").ap()
    wv_d = nc.dram_tensor("w_v", [C, C], BF16, kind="ExternalInput").ap()
    bq_d = nc.dram_tensor("b_qkv", [1, 3 * C], FP32, kind="ExternalInput").ap()
    wo_d = nc.dram_tensor("w_out", [C, C], BF16, kind="ExternalInput").ap()
    bo_d = nc.dram_tensor("b_out", [1, C], FP32, kind="ExternalInput").ap()
    out_d = nc.dram_tensor("out_b", [T, C], FP32, kind="ExternalOutput").ap()

    with tile.TileContext(nc) as tc, ExitStack() as ctx:
        consts = ctx.enter_context(tc.tile_pool(name="consts", bufs=1))
        wpool = ctx.enter_context(tc.tile_pool(name="weights", bufs=1))
        apool = ctx.enter_context(tc.tile_pool(name="acts", bufs=1))
        ppool = ctx.enter_context(tc.tile_pool(name="ppool", bufs=4))
        rbpool = ctx.enter_context(tc.tile_pool(name="rbpool", bufs=2))
        outs = ctx.enter_context(tc.tile_pool(name="outs", bufs=2))
        # PSUM: P1 generic [128,512] (projections), P2 scores+bcast, P3 attn out
        P1 = ctx.enter_context(tc.tile_pool(name="P1", bufs=2, space="PSUM"))
        P2 = ctx.enter_context(tc.tile_pool(name="P2", bufs=2, space="PSUM"))
        P3 = ctx.enter_context(tc.tile_pool(name="P3", bufs=2, space="PSUM"))

        # ---- tiny DMAs + constants FIRST (GpSimd issues DMAs and builds
        # constants in-order; constants must not sit behind the big DMAs) ----
        bqkv_sb = consts.tile([1, C], BF16, tag="bqkv")  # v-part bias only
        nc.gpsimd.dma_start(out=bqkv_sb, in_=bq_d[:, 2 * C:3 * C])
        bout_sb = consts.tile([1, C], BF16, tag="bout")
        nc.gpsimd.dma_start(out=bout_sb, in_=bo_d)
        bqT = consts.tile([128, 16], FP32, tag="bqT")
        nc.sync.dma_start(
            out=bqT,
            in_=bq_d[:, 0:2 * C].rearrange("x (jt p) -> p (x jt)", p=128))

        identity = consts.tile([128, 128], BF16, tag="identity")
        make_identity(nc, identity)
        # diag_mask2[jj, hx, ii] = 0 if ii >= jj else MASK_VAL (both heads)
        diag_mask2 = consts.tile([128, 2, 128], FP32, tag="diag_mask2")
        nc.gpsimd.memset(diag_mask2, 0.0)
        for hx in range(2):
            nc.gpsimd.affine_select(
                out=diag_mask2[:, hx, :], in_=diag_mask2[:, hx, :],
                compare_op=mybir.AluOpType.is_ge, fill=MASK_VAL,
                base=0, channel_multiplier=-1, pattern=[[1, 128]],
            )
        ones_row = consts.tile([1, 512], BF16, tag="ones_row")
        nc.vector.memset(ones_row, 1.0)
        ones64 = consts.tile([1, 64], FP32, tag="ones64")
        nc.vector.memset(ones64, 1.0)

        # ---- persistent activations ----
        # xTall[:, cc, t] == xT chunk cc (merged so transpose casts batch 4x)
        xTall = apool.tile([128, CCH, T], BF16, tag="xTall", name="xTall")
        xT = [xTall[:, cc, :] for cc in range(CCH)]
        # fp8 copy of xT for the DoubleRow qk projection: pair chunk q uses
        # cc=2q (i=0) and cc=2q+1 (i=1) as the in-cell weight/moving pairs
        xT8all = apool.tile([128, CCH, T], FP8, tag="xT8all", name="xT8all")
        qkT = [apool.tile([128, T], BF16, tag=f"qkT{jt}", name=f"qkT{jt}")
               for jt in range(16)]
        vp = [apool.tile([128, H * (D + 1)], BF16, tag=f"vp{t_}", name=f"vp{t_}")
              for t_ in range(TCH)]
        oT = [apool.tile([128, T], BF16, tag=f"oT{hc}", name=f"oT{hc}")
              for hc in range(CCH)]
        partial = [[apool.tile([128, 512], BF16, tag=f"pa{ti}_{hf}",
                               name=f"pa{ti}_{hf}") for hf in range(2)]
                   for ti in range(TCH)]

        # ---- PE warm-up: dummy matmuls during the x DMA (keep the HAM
        # activity window busy so the clock gate reaches 8/8 early) ----
        for w in range(N_WARMUP):
            pd = P3.tile([128, 128], FP32, tag="po", name=f"warm{w}")
            nc.tensor.matmul(out=pd, lhsT=identity, rhs=identity,
                             start=True, stop=True)

        # ---- x load (8 chunks; bufs=8 so no DMA issue gating on slot reuse)
        # then weight DMAs in consumption order, then transposes ----
        with tc.tile_pool(name="xstage", bufs=6) as xstage:
            xs_all = []
            for ti in range(TCH):
                xs = xstage.tile([128, C], BF16, tag="xs", name=f"xs{ti}")
                nc.gpsimd.dma_start(out=xs, in_=x_d[ti * 128:(ti + 1) * 128, :])
                xs_all.append(xs)

            # fp8 q/k weights, host-prepped in pair-interleaved layout:
            # wqk8_sb[k, pos, q, i, j] = W_qkv[256q + 128i + k, 128*jt + j]
            wqk8_sb = wpool.tile([128, 16, 4, 2, 128], FP8, tag="wqk8",
                                 name="wqk8")

            def dma_wq(pos):
                nc.sync.dma_start(
                    out=wqk8_sb[:, pos],
                    in_=wq8_d[:, pos * 1024:(pos + 1) * 1024])

            wv_col = [None] * 2

            def dma_wv(jvt):
                t_ = wpool.tile([128, CCH, 512], BF16, tag=f"wv{jvt}",
                                name=f"wv{jvt}")
                src = wv_d[:, jvt * 512:(jvt + 1) * 512]
                nc.sync.dma_start(
                    out=t_, in_=src.rearrange("(cc p) j -> p cc j", p=128))
                wv_col[jvt] = t_

            dma_wv(0)
            dma_wq(0)
            dma_wq(1)
            dma_wv(1)
            for pos in range(2, 16):
                dma_wq(pos)
            wo_col = wpool.tile([128, CCH, C], BF16, tag="wo", name="wo")
            nc.sync.dma_start(
                out=wo_col, in_=wo_d.rearrange("(cc p) j -> p cc j", p=128))

            # transposes: x[t,c] 128x128 blocks -> xTall[c-chunk, t] as regular
            # matmuls (counts as PE activity); casts batched 4 chunks wide.
            for ti in range(TCH):
                for cc4 in range(0, CCH, 4):
                    pt = P2.tile([128, 4, 128], FP32, tag="ps", name="tp")
                    for k in range(4):
                        cc = cc4 + k
                        nc.tensor.matmul(
                            out=pt[:, k, :],
                            lhsT=xs_all[ti][:, cc * 128:(cc + 1) * 128],
                            rhs=identity, start=True, stop=True)
                    nc.vector.tensor_copy(
                        out=xTall[:, cc4:cc4 + 4, ti * 128:(ti + 1) * 128],
                        in_=pt)
                    nc.scalar.copy(
                        out=xT8all[:, cc4:cc4 + 4, ti * 128:(ti + 1) * 128],
                        in_=pt)

        # gap dummies: keep the PE busy between transposes and v/qk start
        # (waiting on the wv/wq DMA streams)
        for w in range(16):
            pd = P3.tile([128, 512], FP32, tag="po", name=f"gap{w}")
            nc.tensor.matmul(out=pd, lhsT=ones_row[0:1, 0:128],
                             rhs=ones_row[0:1, 0:512], start=True, stop=True)

        # broadcast biases across partitions once (K=1 matmul + copy)
        bvb = consts.tile([128, C], FP32, tag="bvb")
        bob = consts.tile([128, C], FP32, tag="bob")
        for half in range(2):
            sl = slice(half * 512, (half + 1) * 512)
            pb = P1.tile([128, 512], FP32, tag="p1", name="pbias")
            nc.tensor.matmul(
                out=pb, lhsT=ones_row[0:1, 0:128],
                rhs=bqkv_sb[0:1, half * 512:(half + 1) * 512],
                start=True, stop=True)
            nc.vector.tensor_copy(out=bvb[:, sl], in_=pb)
            pb2 = P1.tile([128, 512], FP32, tag="p1", name="pbias2")
            nc.tensor.matmul(out=pb2, lhsT=ones_row[0:1, 0:128],
                             rhs=bout_sb[0:1, sl], start=True, stop=True)
            nc.vector.tensor_copy(out=bob[:, sl], in_=pb2)

        # ones columns of vp (the denominator trick)
        for ti in range(TCH):
            vcol = vp[ti].rearrange("p (h d) -> p h d", h=H)
            nc.vector.memset(vcol[:, :, D:D + 1], 1.0)

        # ---- unit builders ----
        def qk_group(jt, half):
            sl = slice(half * 512, (half + 1) * 512)
            pos = JT_ORDER.index(jt)
            ps = P1.tile([128, 512], FP32, tag="p1", name="psqk")
            for q4 in range(4):
                rhs = xT8all[:, 2 * q4:2 * q4 + 2, sl]
                nc.tensor.matmul(
                    out=ps,
                    lhsT=wqk8_sb[:, pos, q4, :, :],
                    rhs=rhs,
                    start=(q4 == 0), stop=(q4 == 3),
                    perf_mode=mybir.MatmulPerfMode.DoubleRow)
            nc.vector.tensor_scalar_add(
                out=qkT[jt][:, sl], in0=ps, scalar1=bqT[:, jt:jt + 1])

        def v_unit(ti, jvt):
            vcol = vp[ti].rearrange("p (h d) -> p h d", h=H)
            ps = P1.tile([128, 512], FP32, tag="p1", name="psv")
            for cc in range(CCH):
                nc.tensor.matmul(
                    out=ps,
                    lhsT=xT[cc][:, ti * 128:(ti + 1) * 128],
                    rhs=wv_col[jvt][:, cc, :],
                    start=(cc == 0), stop=(cc == CCH - 1))
            nc.vector.tensor_tensor(
                out=vcol[:, jvt * 8:(jvt + 1) * 8, 0:D],
                in0=ps.rearrange("p (h d) -> p h d", h=8),
                in1=bvb[:, jvt * 512:(jvt + 1) * 512].rearrange(
                    "p (h d) -> p h d", h=8),
                op=mybir.AluOpType.add)

        def out_pass1(ti, half):
            # partial output projection over head chunks 0-5 (+bias), bf16
            sl = slice(half * 512, (half + 1) * 512)
            ps = P1.tile([128, 512], FP32, tag="p1", name="pso1")
            for hc in range(6):
                nc.tensor.matmul(
                    out=ps,
                    lhsT=oT[hc][:, ti * 128:(ti + 1) * 128],
                    rhs=wo_col[:, hc, sl],
                    start=(hc == 0), stop=(hc == 5))
            nc.vector.tensor_tensor(
                out=partial[ti][half], in0=ps, in1=bob[:, sl],
                op=mybir.AluOpType.add)

        # ---- upfront: v first half for token blocks 0-3 (heads 0-7 serve
        # head-pairs 0-3), qk for head-pair 0 ----
        for ti in range(4):
            v_unit(ti, 0)
        for jt in (0, 8):
            for half in range(2):
                qk_group(jt, half)

        # ---- filler queue (deadline-ordered) ----
        fillers = []
        fillers += [("v", ti, 0) for ti in range(4, 8)]
        fillers += [("qk", 1, 0), ("qk", 1, 1), ("qk", 9, 0), ("qk", 9, 1)]
        fillers += [("v", 0, 1), ("v", 1, 1)]
        for hp in range(2, 8):
            for jt in (hp, 8 + hp):
                for half in range(2):
                    fillers.append(("qk", jt, half))
            if hp <= 4:
                fillers.append(("v", 2 * (hp - 1), 1))
                fillers.append(("v", 2 * (hp - 1) + 1, 1))
        fillers += [("o1", ti, hf) for ti in range(TCH) for hf in range(2)]
        fil_pos = 0

        def run_filler(n=1):
            nonlocal fil_pos
            for _ in range(n):
                if fil_pos >= len(fillers):
                    return
                f = fillers[fil_pos]
                fil_pos += 1
                if f[0] == "qk":
                    qk_group(f[1], f[2])
                elif f[0] == "v":
                    v_unit(f[1], f[2])
                else:
                    out_pass1(f[1], f[2])

        # filler slot plan: {hp: {it: {jc: count}}}
        def slot_count(hp, it, jc):
            if hp == 0:
                return ({1: 2, 2: 2, 3: 2} if it == 0
                        else {1: 1, 3: 1, 5: 1, 7: 1}).get(jc, 0)
            if hp <= 3:
                return ({1: 1, 2: 1, 3: 1} if it == 0
                        else {1: 1, 3: 1, 5: 1, 7: 1}).get(jc, 0)
            if hp <= 6:
                return ({1: 1, 3: 1} if it == 0
                        else {1: 1, 5: 1}).get(jc, 0)
            return ({1: 1, 2: 1, 3: 1} if it == 0
                    else {1: 2, 3: 2, 5: 2, 7: 2}).get(jc, 0)

        # ---- attention ----
        for hp in range(8):
            h0, h1 = 2 * hp, 2 * hp + 1
            qk_q, qk_k = qkT[hp], qkT[8 + hp]
            for it in range(2):
                njc = 4 * (it + 1)
                po2 = [P3.tile([65, 512], FP32, tag="po", name=f"po{hx}")
                       for hx in range(2)]
                for jc in range(njc):
                    s0 = max(0, jc * 128 - it * 512)
                    ps = P2.tile([128, 2, 512], FP32, tag="ps", name="pss")
                    for hx in range(2):
                        prow = slice(hx * 64, hx * 64 + 64)
                        nc.tensor.matmul(
                            out=ps[:, hx, s0:512],
                            lhsT=qk_k[prow, jc * 128:(jc + 1) * 128],
                            rhs=qk_q[prow, it * 512 + s0:(it + 1) * 512],
                            start=True, stop=True)
                    if jc >= it * 4:  # diagonal block cols [s0, s0+128)
                        nc.vector.tensor_tensor(
                            out=ps[:, :, s0:s0 + 128],
                            in0=ps[:, :, s0:s0 + 128],
                            in1=diag_mask2, op=mybir.AluOpType.add)
                    pT = ppool.tile([128, 2, 512], BF16, tag="pT", name="pT")
                    nc.scalar.activation(
                        out=pT[:, :, s0:512], in_=ps[:, :, s0:512],
                        func=mybir.ActivationFunctionType.Exp, scale=0.125)
                    # PE filler while ACT computes the exp
                    run_filler(slot_count(hp, it, jc))
                    for hx, h in enumerate((h0, h1)):
                        hsl = slice(h * (D + 1), h * (D + 1) + D + 1)
                        nc.tensor.matmul(
                            out=po2[hx][0:65, s0:512],
                            lhsT=vp[jc][:, hsl],
                            rhs=pT[:, hx, s0:512],
                            start=(jc == 0), stop=(jc == njc - 1),
                            skip_group_check=True)
                # normalize: row 64 of po = l = sum_j p
                for hx in range(2):
                    po = po2[hx]
                    prow = slice(hx * 64, hx * 64 + 64)
                    l_sb = rbpool.tile([1, 512], FP32, tag="l", name="l")
                    if hx == 0:  # split the copy across ACT/DVE
                        nc.scalar.copy(out=l_sb, in_=po[64:65, :])
                    else:
                        nc.vector.tensor_copy(out=l_sb, in_=po[64:65, :])
                    plb = P2.tile([64, 512], FP32, tag="ps", name="plb")
                    nc.tensor.matmul(out=plb, lhsT=ones64, rhs=l_sb,
                                     start=True, stop=True)
                    rb = rbpool.tile([64, 512], FP32, tag="rb", name="rb")
                    nc.vector.reciprocal_approx_fast(out=rb, in_=plb)
                    nc.vector.tensor_tensor(
                        out=oT[hp][prow, it * 512:(it + 1) * 512],
                        in0=po[0:64, :], in1=rb, op=mybir.AluOpType.mult)

        # leftover fillers (the last out_pass1 units)
        run_filler(len(fillers) - fil_pos)

        # ---- output projection: remaining head chunks 6-7 + partial ----
        for ti in range(TCH):
            for half in range(2):
                sl = slice(half * 512, (half + 1) * 512)
                ot = outs.tile([128, 512], FP32, tag="ot", name="ot")
                ps = P1.tile([128, 512], FP32, tag="p1", name="pso2")
                for hc in (6, 7):
                    nc.tensor.matmul(
                        out=ps,
                        lhsT=oT[hc][:, ti * 128:(ti + 1) * 128],
                        rhs=wo_col[:, hc, sl],
                        start=(hc == 6), stop=(hc == 7))
                nc.vector.tensor_tensor(
                    out=ot, in0=ps, in1=partial[ti][half],
                    op=mybir.AluOpType.add)
                nc.sync.dma_start(
                    out=out_d[ti * 128:(ti + 1) * 128, sl], in_=ot)

    nc.compile()
    nc.finalize()
    return nc


_CACHE = {}


def make_in_maps(x, W_qkv, b_qkv, W_out, b_out):
    import ml_dtypes
    bf16 = ml_dtypes.bfloat16
    fp8 = ml_dtypes.float8_e4m3  # IEEE e4m3 (max 240) == TRN FP8_EXP4
    x = np.ascontiguousarray(np.asarray(x, np.float32).astype(bf16))
    W_qkv = np.asarray(W_qkv, np.float32)
    # q/k weights: bf16 round then fp8, pair-interleaved [k, pos, q, i, j]
    w8 = W_qkv[:, :2 * C].astype(bf16).astype(np.float32)
    w8 = w8.reshape(4, 2, 128, 16, 128)          # [q, i, k, jt, j]
    w8 = w8[:, :, :, JT_ORDER, :]                # jt -> consumption pos
    w8 = w8.transpose(2, 3, 0, 1, 4)             # [k, pos, q, i, j]
    w8 = np.ascontiguousarray(w8.reshape(128, -1).astype(fp8))
    wv = np.ascontiguousarray(W_qkv[:, 2 * C:].astype(bf16))
    wo = np.ascontiguousarray(np.asarray(W_out, np.float32).astype(bf16))
    bq = np.ascontiguousarray(np.asarray(b_qkv, np.float32).reshape(1, -1))
    bo = np.ascontiguousarray(np.asarray(b_out, np.float32).reshape(1, -1))
    return [
        {"x_b": x[i], "w_qk8": w8, "w_v": wv, "b_qkv": bq, "w_out": wo,
         "b_out": bo}
        for i in range(N_CORES)
    ]


def kernel(x, W_qkv, b_qkv, W_out, b_out):
    if "nc" not in _CACHE:
        _CACHE["nc"] = build_nc()
    nc = _CACHE["nc"]
    in_maps = make_in_maps(x, W_qkv, b_qkv, W_out, b_out)
    res = bass_utils.run_bass_kernel_spmd(nc, in_maps, core_ids=list(range(N_CORES)))
    return np.stack([r["out_b"] for r in res.results]).astype(np.float32)


# revision 36
# speedup vs baseline: 1.1755x; 1.1755x over previous
# Causal self-attention (B=8, T=1024, C=1024, H=16, D=64) on 8 trn2 NeuronCores.
# Sharding: data-parallel over batch — core i computes batch element i entirely
# (weights replicated, no collectives).
#
# v3 schedule notes: the PE HAM clock gate re-throttles to 1.2 GHz after ~3.4us
# of contiguous idle, and transpose-mode matmuls don't count as PE activity.
#   - constants (identity/masks) are built on GpSimd BEFORE the big DMA issue
#     train (GpSimd both issues DMAs and builds constants, in order),
#   - dummy identity matmuls warm the PE during the initial x DMA,
#   - x transposes are regular matmuls (lhsT=x_block, rhs=identity) with
#     4-wide batched PSUM->SBUF casts,
#   - qk-projection groups, second-half v-projection units and partial output
#     projection (head chunks 0-5) are distributed through the attention phase
#     from a deadline-ordered filler queue so the PE never idles long enough
#     to re-throttle,
#   - softmax denominator: reciprocal straight off PSUM row 64 (no l copy),
#     K=1 matmul broadcast, DVE multiply.
#
# Per-core pipeline (all matmuls bf16 inputs, fp32 PSUM accumulation):
#   0. x [T,C] --cast-dma--> bf16, transpose via PE -> xT [C,T]
#   1. v[t,j] per half jvt (heads 8*jvt..) with a ones column (width 65) so
#      attn@v also yields the softmax denominator in row 64.
#   2. per head-pair hp: qkT[j,t] (lhsT=W column slice, rhs=xT), then per
#      (head, 512-query tile): scores sT[j,i] (K=64, two heads in row groups
#      0/64 run concurrently), merged diag-block mask add (DVE), exp on ACT
#      (scale=1/8) -> pT bf16, attn@v (M=65), reciprocal+bcast+mult -> oT.
#   3. out[t,c]: lhsT=oT chunk, rhs=W_out; head chunks 0-5 pre-accumulated
#      into bf16 partials during hp6/7, chunks 6-7 + partial + bias at the end.

import numpy as np
from contextlib import ExitStack

import concourse.bass as bass
import concourse.bacc as bacc
import concourse.mybir as mybir
import concourse.tile as tile
from concourse import bass_utils
from concourse.masks import make_identity

FP32 = mybir.dt.float32
BF16 = mybir.dt.bfloat16
FP8 = mybir.dt.float8e4
# wqk8 jt consumption order (head-pair major)
JT_ORDER = [0, 8, 1, 9, 2, 10, 3, 11, 4, 12, 5, 13, 6, 14, 7, 15]

B, T, C = 8, 1024, 1024
H, D = 16, 64
N_CORES = 8
MASK_VAL = -1e4  # pre-scale additive mask; exp(0.125 * (s + MASK_VAL)) == 0.0
CCH = C // 128   # 8 contraction chunks of 128
TCH = T // 128   # 8 token chunks of 128
N_WARMUP = 40    # dummy matmuls to warm the PE clock gate during x DMA


def build_nc():
    nc = bacc.Bacc("TRN2", debug=False, num_devices=N_CORES)

    x_d = nc.dram_tensor("x_b", [T, C], BF16, kind="ExternalInput").ap()
    wq8_d = nc.dram_tensor("w_qk8", [128, 16 * 1024], FP8, kind="ExternalInput").ap()
    wv_d = nc.dram_tensor("w_v", [C, C], BF16, kind="ExternalInput").ap()
    bq_d = nc.dram_tensor("b_qkv", [1, 3 * C], FP32, kind="ExternalInput").ap()
    wo_d = nc.dram_tensor("w_out", [C, C], BF16, kind="ExternalInput").ap()
    bo_d = nc.dram_tensor("b_out", [1, C], FP32, kind="ExternalInput").ap()
    out_d = nc.dram_tensor("out_b", [T, C], FP32, kind="ExternalOutput").ap()

    with tile.TileContext(nc) as tc, ExitStack() as ctx:
        consts = ctx.enter_context(tc.tile_pool(name="consts", bufs=1))
        wpool = ctx.enter_context(tc.tile_pool(name="weights", bufs=1))
        apool = ctx.enter_context(tc.tile_pool(name="acts", bufs=1))
        ppool = ctx.enter_context(tc.tile_pool(name="ppool", bufs=4))
        rbpool = ctx.enter_context(tc.tile_pool(name="rbpool", bufs=2))
        outs = ctx.enter_context(tc.tile_pool(name="outs", bufs=2))
        # PSUM: P1 generic [128,512] (projections), P2 scores+bcast, P3 attn out
        P1 = ctx.enter_context(tc.tile_pool(name="P1", bufs=2, space="PSUM"))
        P2 = ctx.enter_context(tc.tile_pool(name="P2", bufs=2, space="PSUM"))
        P3 = ctx.enter_context(tc.tile_pool(name="P3", bufs=2, space="PSUM"))

        # ---- tiny DMAs + constants FIRST (GpSimd issues DMAs and builds
        # constants in-order; constants must not sit behind the big DMAs) ----
        bqkv_sb = consts.tile([1, C], BF16, tag="bqkv")  # v-part bias only
        nc.gpsimd.dma_start(out=bqkv_sb, in_=bq_d[:, 2 * C:3 * C])
        bout_sb = consts.tile([1, C], BF16, tag="bout")
        nc.gpsimd.dma_start(out=bout_sb, in_=bo_d)
        bqT = consts.tile([128, 16], FP32, tag="bqT")
        nc.sync.dma_start(
            out=bqT,
            in_=bq_d[:, 0:2 * C].rearrange("x (jt p) -> p (x jt)", p=128))

        identity = consts.tile([128, 128], BF16, tag="identity")
        make_identity(nc, identity)
        # diag_mask2[jj, hx, ii] = 0 if ii >= jj else MASK_VAL (both heads)
        diag_mask2 = consts.tile([128, 2, 128], FP32, tag="diag_mask2")
        nc.gpsimd.memset(diag_mask2, 0.0)
        for hx in range(2):
            nc.gpsimd.affine_select(
                out=diag_mask2[:, hx, :], in_=diag_mask2[:, hx, :],
                compare_op=mybir.AluOpType.is_ge, fill=MASK_VAL,
                base=0, channel_multiplier=-1, pattern=[[1, 128]],
            )
        ones_row = consts.tile([1, 512], BF16, tag="ones_row")
        nc.vector.memset(ones_row, 1.0)
        ones64 = consts.tile([1, 64], FP32, tag="ones64")
        nc.vector.memset(ones64, 1.0)

        # ---- persistent activations ----
        # xTall[:, cc, t] == xT chunk cc (merged so transpose casts batch 4x)
        xTall = apool.tile([128, CCH, T], BF16, tag="xTall", name="xTall")
        xT = [xTall[:, cc, :] for cc in range(CCH)]
        # fp8 copy of xT for the DoubleRow qk projection: pair chunk q uses
        # cc=2q (i=0) and cc=2q+1 (i=1) as the in-cell weight/moving pairs
        xT8all = apool.tile([128, CCH, T], FP8, tag="xT8all", name="xT8all")
        qkT = [apool.tile([128, T], BF16, tag=f"qkT{jt}", name=f"qkT{jt}")
               for jt in range(16)]
        vp = [apool.tile([128, H * (D + 1)], BF16, tag=f"vp{t_}", name=f"vp{t_}")
              for t_ in range(TCH)]
        oT = [apool.tile([128, T], BF16, tag=f"oT{hc}", name=f"oT{hc}")
              for hc in range(CCH)]
        partial = [[apool.tile([128, 512], BF16, tag=f"pa{ti}_{hf}",
                               name=f"pa{ti}_{hf}") for hf in range(2)]
                   for ti in range(TCH)]

        # ---- PE warm-up: dummy matmuls during the x DMA (keep the HAM
        # activity window busy so the clock gate reaches 8/8 early) ----
        for w in range(N_WARMUP):
            pd = P3.tile([128, 128], FP32, tag="po", name=f"warm{w}")
            nc.tensor.matmul(out=pd, lhsT=identity, rhs=identity,
                             start=True, stop=True)

        # ---- x load (8 chunks; bufs=8 so no DMA issue gating on slot reuse)
        # then weight DMAs in consumption order, then transposes ----
        with tc.tile_pool(name="xstage", bufs=6) as xstage:
            xs_all = []
            for ti in range(TCH):
                xs = xstage.tile([128, C], BF16, tag="xs", name=f"xs{ti}")
                nc.gpsimd.dma_start(out=xs, in_=x_d[ti * 128:(ti + 1) * 128, :])
                xs_all.append(xs)

            # fp8 q/k weights, host-prepped in pair-interleaved layout:
            # wqk8_sb[k, pos, q, i, j] = W_qkv[256q + 128i + k, 128*jt + j]
            wqk8_sb = wpool.tile([128, 16, 4, 2, 128], FP8, tag="wqk8",
                                 name="wqk8")

            def dma_wq(pos):
                nc.sync.dma_start(
                    out=wqk8_sb[:, pos],
                    in_=wq8_d[:, pos * 1024:(pos + 1) * 1024])

            wv_col = [None] * 2

            def dma_wv(jvt):
                t_ = wpool.tile([128, CCH, 512], BF16, tag=f"wv{jvt}",
                                name=f"wv{jvt}")
                src = wv_d[:, jvt * 512:(jvt + 1) * 512]
                nc.sync.dma_start(
                    out=t_, in_=src.rearrange("(cc p) j -> p cc j", p=128))
                wv_col[jvt] = t_

            dma_wv(0)
            dma_wq(0)
            dma_wq(1)
            dma_wv(1)
            for pos in range(2, 16):
                dma_wq(pos)
            wo_col = wpool.tile([128, CCH, C], BF16, tag="wo", name="wo")
            nc.sync.dma_start(
                out=wo_col, in_=wo_d.rearrange("(cc p) j -> p cc j", p=128))

            # transposes: x[t,c] 128x128 blocks -> xTall[c-chunk, t] as regular
            # matmuls (counts as PE activity); casts batched 4 chunks wide.
            for ti in range(TCH):
                for cc4 in range(0, CCH, 4):
                    pt = P2.tile([128, 4, 128], FP32, tag="ps", name="tp")
                    for k in range(4):
                        cc = cc4 + k
                        nc.tensor.matmul(
                            out=pt[:, k, :],
                            lhsT=xs_all[ti][:, cc * 128:(cc + 1) * 128],
                            rhs=identity, start=True, stop=True)
                    nc.vector.tensor_copy(
                        out=xTall[:, cc4:cc4 + 4, ti * 128:(ti + 1) * 128],
                        in_=pt)
                    nc.scalar.copy(
                        out=xT8all[:, cc4:cc4 + 4, ti * 128:(ti + 1) * 128],
                        in_=pt)

        # gap dummies: keep the PE busy between transposes and v/qk start
        # (waiting on the wv/wq DMA streams)
        for w in range(16):
            pd = P3.tile([128, 512], FP32, tag="po", name=f"gap{w}")
            nc.tensor.matmul(out=pd, lhsT=ones_row[0:1, 0:128],
                             rhs=ones_row[0:1, 0:512], start=True, stop=True)

        # broadcast biases across partitions once (K=1 matmul + copy)
        bvb = consts.tile([128, C], FP32, tag="bvb")
        bob = consts.tile([128, C], FP32, tag="bob")
        for half in range(2):
            sl = slice(half * 512, (half + 1) * 512)
            pb = P1.tile([128, 512], FP32, tag="p1", name="pbias")
            nc.tensor.matmul(
                out=pb, lhsT=ones_row[0:1, 0:128],
                rhs=bqkv_sb[0:1, half * 512:(half + 1) * 512],
                start=True, stop=True)
            nc.vector.tensor_copy(out=bvb[:, sl], in_=pb)
            pb2 = P1.tile([128, 512], FP32, tag="p1", name="pbias2")
            nc.tensor.matmul(out=pb2, lhsT=ones_row[0:1, 0:128],
                             rhs=bout_sb[0:1, sl], start=True, stop=True)
            nc.vector.tensor_copy(out=bob[:, sl], in_=pb2)

        # ones columns of vp (the denominator trick)
        for ti in range(TCH):
            vcol = vp[ti].rearrange("p (h d) -> p h d", h=H)
            nc.vector.memset(vcol[:, :, D:D + 1], 1.0)

        # ---- unit builders ----
        def qk_group(jt, half):
            sl = slice(half * 512, (half + 1) * 512)
            pos = JT_ORDER.index(jt)
            ps = P1.tile([128, 512], FP32, tag="p1", name="psqk")
            for q4 in range(4):
                rhs = xT8all[:, 2 * q4:2 * q4 + 2, sl]
                nc.tensor.matmul(
                    out=ps,
                    lhsT=wqk8_sb[:, pos, q4, :, :],
                    rhs=rhs,
                    start=(q4 == 0), stop=(q4 == 3),
                    perf_mode=mybir.MatmulPerfMode.DoubleRow)
            nc.vector.tensor_scalar_add(
                out=qkT[jt][:, sl], in0=ps, scalar1=bqT[:, jt:jt + 1])

        def v_unit(ti, jvt):
            vcol = vp[ti].rearrange("p (h d) -> p h d", h=H)
            ps = P1.tile([128, 512], FP32, tag="p1", name="psv")
            for cc in range(CCH):
                nc.tensor.matmul(
                    out=ps,
                    lhsT=xT[cc][:, ti * 128:(ti + 1) * 128],
                    rhs=wv_col[jvt][:, cc, :],
                    start=(cc == 0), stop=(cc == CCH - 1))
            nc.vector.tensor_tensor(
                out=vcol[:, jvt * 8:(jvt + 1) * 8, 0:D],
                in0=ps.rearrange("p (h d) -> p h d", h=8),
                in1=bvb[:, jvt * 512:(jvt + 1) * 512].rearrange(
                    "p (h d) -> p h d", h=8),
                op=mybir.AluOpType.add)

        def out_pass1(ti, half):
            # partial output projection over head chunks 0-5 (+bias), bf16
            sl = slice(half * 512, (half + 1) * 512)
            ps = P1.tile([128, 512], FP32, tag="p1", name="pso1")
            for hc in range(6):
                nc.tensor.matmul(
                    out=ps,
                    lhsT=oT[hc][:, ti * 128:(ti + 1) * 128],
                    rhs=wo_col[:, hc, sl],
                    start=(hc == 0), stop=(hc == 5))
            nc.vector.tensor_tensor(
                out=partial[ti][half], in0=ps, in1=bob[:, sl],
                op=mybir.AluOpType.add)

        # ---- upfront: v first half for token blocks 0-3 (heads 0-7 serve
        # head-pairs 0-3), qk for head-pair 0 ----
        for ti in range(4):
            v_unit(ti, 0)
        for jt in (0, 8):
            for half in range(2):
                qk_group(jt, half)

        # ---- filler queue (deadline-ordered; fp8 qk groups are ~1us,
        # v units ~1.8us, o1 ~1.3us) ----
        def qk4(hp):
            return [("qk", jt, half) for jt in (hp, 8 + hp)
                    for half in range(2)]
        fillers = []
        fillers += [("v", ti, 0) for ti in range(4, 8)]
        fillers += qk4(1) + [("v", 0, 1), ("v", 1, 1)]
        fillers += qk4(2) + [("v", 2, 1), ("v", 3, 1)]
        fillers += qk4(3) + qk4(4)
        fillers += [("v", 4, 1), ("v", 5, 1), ("v", 6, 1), ("v", 7, 1)]
        fillers += qk4(5) + qk4(6) + qk4(7)
        fillers += [("o1", ti, hf) for ti in range(TCH) for hf in range(2)]
        fil_pos = 0

        def run_filler(n=1):
            nonlocal fil_pos
            for _ in range(n):
                if fil_pos >= len(fillers):
                    return
                f = fillers[fil_pos]
                fil_pos += 1
                if f[0] == "qk":
                    qk_group(f[1], f[2])
                elif f[0] == "v":
                    v_unit(f[1], f[2])
                else:
                    out_pass1(f[1], f[2])

        # filler slot plan: {hp: {it: {jc: count}}}
        def slot_count(hp, it, jc):
            if hp == 0:
                return ({1: 2, 2: 2, 3: 2} if it == 0
                        else {1: 1, 3: 1, 5: 1, 7: 1}).get(jc, 0)
            if hp <= 3:
                return ({1: 1, 2: 1, 3: 1} if it == 0
                        else {1: 1, 3: 1, 5: 1, 7: 1}).get(jc, 0)
            if hp <= 6:
                return ({1: 1, 3: 1} if it == 0
                        else {1: 1, 5: 1}).get(jc, 0)
            return ({1: 1, 2: 1, 3: 1} if it == 0
                    else {1: 2, 3: 2, 5: 2, 7: 2}).get(jc, 0)

        # ---- attention ----
        for hp in range(8):
            h0, h1 = 2 * hp, 2 * hp + 1
            qk_q, qk_k = qkT[hp], qkT[8 + hp]
            for it in range(2):
                njc = 4 * (it + 1)
                po2 = [P3.tile([65, 512], FP32, tag="po", name=f"po{hx}")
                       for hx in range(2)]
                for jc in range(njc):
                    s0 = max(0, jc * 128 - it * 512)
                    ps = P2.tile([128, 2, 512], FP32, tag="ps", name="pss")
                    for hx in range(2):
                        prow = slice(hx * 64, hx * 64 + 64)
                        nc.tensor.matmul(
                            out=ps[:, hx, s0:512],
                            lhsT=qk_k[prow, jc * 128:(jc + 1) * 128],
                            rhs=qk_q[prow, it * 512 + s0:(it + 1) * 512],
                            start=True, stop=True)
                    if jc >= it * 4:  # diagonal block cols [s0, s0+128)
                        nc.vector.tensor_tensor(
                            out=ps[:, :, s0:s0 + 128],
                            in0=ps[:, :, s0:s0 + 128],
                            in1=diag_mask2, op=mybir.AluOpType.add)
                    pT = ppool.tile([128, 2, 512], BF16, tag="pT", name="pT")
                    nc.scalar.activation(
                        out=pT[:, :, s0:512], in_=ps[:, :, s0:512],
                        func=mybir.ActivationFunctionType.Exp, scale=0.125)
                    # PE filler while ACT computes the exp
                    run_filler(slot_count(hp, it, jc))
                    for hx, h in enumerate((h0, h1)):
                        hsl = slice(h * (D + 1), h * (D + 1) + D + 1)
                        nc.tensor.matmul(
                            out=po2[hx][0:65, s0:512],
                            lhsT=vp[jc][:, hsl],
                            rhs=pT[:, hx, s0:512],
                            start=(jc == 0), stop=(jc == njc - 1),
                            skip_group_check=True)
                # normalize: row 64 of po = l = sum_j p
                for hx in range(2):
                    po = po2[hx]
                    prow = slice(hx * 64, hx * 64 + 64)
                    l_sb = rbpool.tile([1, 512], FP32, tag="l", name="l")
                    if hx == 0:  # split the copy across ACT/DVE
                        nc.scalar.copy(out=l_sb, in_=po[64:65, :])
                    else:
                        nc.vector.tensor_copy(out=l_sb, in_=po[64:65, :])
                    plb = P2.tile([64, 512], FP32, tag="ps", name="plb")
                    nc.tensor.matmul(out=plb, lhsT=ones64, rhs=l_sb,
                                     start=True, stop=True)
                    rb = rbpool.tile([64, 512], FP32, tag="rb", name="rb")
                    nc.vector.reciprocal_approx_fast(out=rb, in_=plb)
                    nc.vector.tensor_tensor(
                        out=oT[hp][prow, it * 512:(it + 1) * 512],
                        in0=po[0:64, :], in1=rb, op=mybir.AluOpType.mult)

        # leftover fillers (the last out_pass1 units)
        run_filler(len(fillers) - fil_pos)

        # ---- output projection: remaining head chunks 6-7 + partial ----
        for ti in range(TCH):
            for half in range(2):
                sl = slice(half * 512, (half + 1) * 512)
                ot = outs.tile([128, 512], FP32, tag="ot", name="ot")
                ps = P1.tile([128, 512], FP32, tag="p1", name="pso2")
                for hc in (6, 7):
                    nc.tensor.matmul(
                        out=ps,
                        lhsT=oT[hc][:, ti * 128:(ti + 1) * 128],
                        rhs=wo_col[:, hc, sl],
                        start=(hc == 6), stop=(hc == 7))
                nc.vector.tensor_tensor(
                    out=ot, in0=ps, in1=partial[ti][half],
                    op=mybir.AluOpType.add)
                nc.sync.dma_start(
                    out=out_d[ti * 128:(ti + 1) * 128, sl], in_=ot)

    nc.compile()
    nc.finalize()
    return nc


_CACHE = {}


def make_in_maps(x, W_qkv, b_qkv, W_out, b_out):
    import ml_dtypes
    bf16 = ml_dtypes.bfloat16
    fp8 = ml_dtypes.float8_e4m3  # IEEE e4m3 (max 240) == TRN FP8_EXP4
    x = np.ascontiguousarray(np.asarray(x, np.float32).astype(bf16))
    W_qkv = np.asarray(W_qkv, np.float32)
    # q/k weights: bf16 round then fp8, pair-interleaved [k, pos, q, i, j]
    w8 = W_qkv[:, :2 * C].astype(bf16).astype(np.float32)
    w8 = w8.reshape(4, 2, 128, 16, 128)          # [q, i, k, jt, j]
    w8 = w8[:, :, :, JT_ORDER, :]                # jt -> consumption pos
    w8 = w8.transpose(2, 3, 0, 1, 4)             # [k, pos, q, i, j]
    w8 = np.ascontiguousarray(w8.reshape(128, -1).astype(fp8))
    wv = np.ascontiguousarray(W_qkv[:, 2 * C:].astype(bf16))
    wo = np.ascontiguousarray(np.asarray(W_out, np.float32).astype(bf16))
    bq = np.ascontiguousarray(np.asarray(b_qkv, np.float32).reshape(1, -1))
    bo = np.ascontiguousarray(np.asarray(b_out, np.float32).reshape(1, -1))
    return [
        {"x_b": x[i], "w_qk8": w8, "w_v": wv, "b_qkv": bq, "w_out": wo,
         "b_out": bo}
        for i in range(N_CORES)
    ]


def kernel(x, W_qkv, b_qkv, W_out, b_out):
    if "nc" not in _CACHE:
        _CACHE["nc"] = build_nc()
    nc = _CACHE["nc"]
    in_maps = make_in_maps(x, W_qkv, b_qkv, W_out, b_out)
    res = bass_utils.run_bass_kernel_spmd(nc, in_maps, core_ids=list(range(N_CORES)))
    return np.stack([r["out_b"] for r in res.results]).astype(np.float32)
